# revision 3
# baseline (speedup 1.0000x reference)
"""LRNetLinear forward on 8 Trainium2 NeuronCores (tensor-parallel over out_features).

Math (per reference):
  3-way softmax over logits [theta_neg, 0, theta_pos] (shift-free: |theta|<=~80 is safe in fp32):
    e_n = exp(theta_neg); e_p = exp(theta_pos); Z = 1 + e_n + e_p; r = 1/Z
    diff = (e_p - e_n) * r
    w_mean = diff * sc                      (sc = scales_exp, constant over 128-wide i-blocks)
    w_var  = (1 - r - diff^2) * sc^2        (since p_pos + p_neg = 1 - r)
  mu = x @ w_mean.T ; s2 = (x*x) @ w_var.T ; out = mu + sqrt(s2 + 1e-8) * eps
  (sqrt via exp(0.5*ln(.)) so every ACT function lives in one table set)

Per-core shard: O_s = 4096/8 = 512 output features. x replicated; out gathered on host.
Matmuls run in float32r (full PE rate, ~1.5e-4 rel err); operands are rounded to
float32r by the PSUM-evacuation copies. x and weights are transposed on-chip with
PE identity-matmul transposes (contraction dim must sit on partitions).
"""
import sys

if "/opt/trn_rl_repo" not in sys.path:
    sys.path.insert(0, "/opt/trn_rl_repo")

import numpy as np

import concourse.bass as bass
import concourse.bacc as bacc
import concourse.mybir as mybir
import concourse.tile as tile
from concourse.bass_utils import run_bass_kernel_spmd
from concourse.masks import make_identity

N_CORES = 8
B = 4096
I = 4096
O = 4096
OS = O // N_CORES          # 512 out features per core
KT = I // 128              # 32 contraction tiles
BT = B // 128              # 32 token tiles
OJ = OS // 128             # 4 o-tiles in weight prep
F32 = mybir.dt.float32
F32R = mybir.dt.float32r
PSUM = bass.MemorySpace.PSUM

_CACHE = {}


def build():
    AF = mybir.ActivationFunctionType
    OP = mybir.AluOpType
    nc = bacc.Bacc("TRN2", target_bir_lowering=False, debug=False, num_devices=N_CORES)
    x_d = nc.dram_tensor("x", [B, I], F32, kind="ExternalInput").ap()
    tn_d = nc.dram_tensor("tn", [OS, I], F32, kind="ExternalInput").ap()
    tp_d = nc.dram_tensor("tp", [OS, I], F32, kind="ExternalInput").ap()
    scs_d = nc.dram_tensor("scs", [OS, KT], F32, kind="ExternalInput").ap()
    eps_d = nc.dram_tensor("eps", [B, OS], F32, kind="ExternalInput").ap()
    out_d = nc.dram_tensor("out", [B, OS], F32, kind="ExternalOutput").ap()

    with tile.TileContext(nc) as tc:
        with tc.tile_pool(name="const", bufs=1) as cp:
            ident = cp.tile([128, 128], F32)
            make_identity(nc, ident)
            # scs[o, k] -> [p, j, k] with o = j*128 + p
            scs_t = cp.tile([128, OJ, KT], F32)
            nc.sync.dma_start(scs_t, scs_d.rearrange("(j p) k -> p j k", p=128))
            sc2 = cp.tile([128, OJ, KT], F32)
            nc.scalar.square(sc2, scs_t)
            nsc2 = cp.tile([128, OJ, KT], F32)
            nc.vector.tensor_scalar_mul(nsc2, sc2, -1.0)
            b1e8 = cp.tile([128, 1], F32)
            nc.vector.memset(b1e8, 1e-8)
            # transposed weights [i_in, k, o], rounded to f32r at evacuation
            wmT = cp.tile([128, KT, OS], F32R)
            wvT = cp.tile([128, KT, OS], F32R)

            # ---------------- weight prep ----------------
            CH = 1024            # i-chunk
            NCH = I // CH        # 4 chunks per o-tile
            SB = CH // 128       # 8 sub-blocks per chunk
            with (
                tc.tile_pool(name="wprep", bufs=2) as wp,
                tc.tile_pool(name="wpsum", bufs=2, space=PSUM) as wps,
            ):
                for j in range(OJ):
                    for c in range(NCH):
                        i0 = c * CH
                        tn_t = wp.tile([128, CH], F32, tag="t0", name="tn_t")
                        nc.sync.dma_start(tn_t, tn_d[128 * j:128 * (j + 1), i0:i0 + CH])
                        tp_t = wp.tile([128, CH], F32, tag="t1", name="tp_t")
                        nc.sync.dma_start(tp_t, tp_d[128 * j:128 * (j + 1), i0:i0 + CH])
                        en = wp.tile([128, CH], F32, tag="t2", name="en")
                        nc.scalar.activation(en, tn_t, AF.Exp)
                        ep = wp.tile([128, CH], F32, tag="t3", name="ep")
                        nc.scalar.activation(ep, tp_t, AF.Exp)
                        Z = wp.tile([128, CH], F32, tag="t0", name="Z")
                        nc.vector.scalar_tensor_tensor(Z, ep, 1.0, en, op0=OP.add, op1=OP.add)
                        d = wp.tile([128, CH], F32, tag="t1", name="d")
                        nc.vector.tensor_sub(d, ep, en)
                        v = wp.tile([128, CH], F32, tag="t2", name="v")
                        nc.scalar.activation(v, Z, AF.Ln)
                        r = wp.tile([128, CH], F32, tag="t3", name="r")
                        nc.scalar.activation(r, v, AF.Exp, scale=-1.0)
                        diff = wp.tile([128, CH], F32, tag="t0", name="diff")
                        nc.vector.tensor_mul(diff, d, r)
                        wm = wp.tile([128, CH], F32, tag="w0", name="wm")
                        for s in range(SB):
                            k = (i0 + 128 * s) // 128
                            nc.vector.tensor_scalar_mul(
                                wm[:, 128 * s:128 * (s + 1)],
                                diff[:, 128 * s:128 * (s + 1)],
                                scs_t[:, j, k:k + 1],
                            )
                        d2 = wp.tile([128, CH], F32, tag="t1", name="d2")
                        nc.scalar.square(d2, diff)
                        q = wp.tile([128, CH], F32, tag="t2", name="q")
                        nc.vector.tensor_add(q, d2, r)
                        wv = wp.tile([128, CH], F32, tag="w1", name="wv")
                        for s in range(SB):
                            k = (i0 + 128 * s) // 128
                            # (q * -sc^2) + sc^2 = sc^2 * (1 - q)
                            nc.vector.tensor_scalar(
                                wv[:, 128 * s:128 * (s + 1)],
                                q[:, 128 * s:128 * (s + 1)],
                                nsc2[:, j, k:k + 1],
                                sc2[:, j, k:k + 1],
                                op0=OP.mult,
                                op1=OP.add,
                            )
                        for g in range(SB // 4):
                            k0 = i0 // 128 + 4 * g
                            pm = wps.tile([128, 4, 128], F32, tag="pm", name="pm")
                            pv = wps.tile([128, 4, 128], F32, tag="pv", name="pv")
                            for s4 in range(4):
                                s = 4 * g + s4
                                nc.tensor.transpose(pm[:, s4, :], wm[:, 128 * s:128 * (s + 1)], ident)
                                nc.tensor.transpose(pv[:, s4, :], wv[:, 128 * s:128 * (s + 1)], ident)
                            nc.vector.tensor_copy(wmT[:, k0:k0 + 4, 128 * j:128 * (j + 1)], pm)
                            nc.scalar.copy(wvT[:, k0:k0 + 4, 128 * j:128 * (j + 1)], pv)

            # ---------------- main loop over token tiles ----------------
            XCH = 1024
            NXCH = I // XCH
            with (
                tc.tile_pool(name="xnat", bufs=4) as xnp,
                tc.tile_pool(name="xtp", bufs=8) as xtp,
                tc.tile_pool(name="x2p", bufs=8) as x2p,
                tc.tile_pool(name="epsp", bufs=2) as epp,
                tc.tile_pool(name="epip", bufs=4) as eip,
                tc.tile_pool(name="xpsum", bufs=3, space=PSUM) as xps,
                tc.tile_pool(name="opsum", bufs=2, space=PSUM) as ops,
            ):
                def load_x(t):
                    tiles = []
                    for h in range(NXCH):
                        xn = xnp.tile([128, XCH], F32, tag="xn", name="xn")
                        nc.sync.dma_start(xn, x_d[128 * t:128 * (t + 1), XCH * h:XCH * (h + 1)])
                        tiles.append(xn)
                    return tiles

                def make_pack(xn_tiles, g):
                    # pack g: i in [512g, 512g+512) -> xT/x2T [128, 4, 128] (f32r)
                    h, off = divmod(512 * g, XCH)
                    ps = xps.tile([128, 4, 128], F32, tag="xp", name="ps")
                    for s4 in range(4):
                        nc.tensor.transpose(
                            ps[:, s4, :], xn_tiles[h][:, off + 128 * s4:off + 128 * (s4 + 1)], ident
                        )
                    xt_g = xtp.tile([128, 4, 128], F32R, tag="xt", name="xt_g")
                    nc.vector.tensor_copy(xt_g, ps)
                    x2_g = x2p.tile([128, 4, 128], F32R, tag="x2", name="x2_g")
                    nc.scalar.square(x2_g, ps)
                    return xt_g, x2_g

                xn = load_x(0)
                packs = [make_pack(xn, g) for g in range(8)]
                for t in range(BT):
                    eps_t = epp.tile([128, OS], F32, tag="ep", name="eps_t")
                    nc.sync.dma_start(eps_t, eps_d[128 * t:128 * (t + 1), :])
                    xn_next = load_x(t + 1) if t + 1 < BT else None
                    pmu = ops.tile([128, OS], F32, tag="pm", name="pmu")
                    pvar = ops.tile([128, OS], F32, tag="pv", name="pvar")
                    next_packs = [None] * 8
                    for k in range(KT):
                        g, s4 = divmod(k, 4)
                        xt_g, x2_g = packs[g]
                        nc.tensor.matmul(pmu, xt_g[:, s4, :], wmT[:, k, :],
                                         start=(k == 0), stop=(k == KT - 1))
                        nc.tensor.matmul(pvar, x2_g[:, s4, :], wvT[:, k, :],
                                         start=(k == 0), stop=(k == KT - 1))
                        if s4 == 3 and xn_next is not None:
                            next_packs[g] = make_pack(xn_next, g)
                    u = eip.tile([128, OS], F32, tag="epi", name="u")
                    nc.scalar.activation(u, pvar, AF.Ln, bias=b1e8)
                    sig = eip.tile([128, OS], F32, tag="epi", name="sig")
                    nc.scalar.activation(sig, u, AF.Exp, scale=0.5)
                    prod = eip.tile([128, OS], F32, tag="epi", name="prod")
                    nc.vector.tensor_mul(prod, sig, eps_t)
                    outt = eip.tile([128, OS], F32, tag="epi", name="outt")
                    nc.vector.tensor_add(outt, pmu, prod)
                    nc.sync.dma_start(out_d[128 * t:128 * (t + 1), :], outt)
                    if xn_next is not None:
                        packs = next_packs

    nc.compile()
    return nc


def _get_nc():
    if "nc" not in _CACHE:
        _CACHE["nc"] = build()
    return _CACHE["nc"]


def kernel(x, theta_neg, theta_pos, scales_exp, eps):
    nc = _get_nc()
    x = np.ascontiguousarray(np.asarray(x, np.float32))
    in_maps = []
    for j in range(N_CORES):
        sl = slice(OS * j, OS * (j + 1))
        in_maps.append({
            "x": x,
            "tn": np.ascontiguousarray(np.asarray(theta_neg, np.float32)[sl]),
            "tp": np.ascontiguousarray(np.asarray(theta_pos, np.float32)[sl]),
            "scs": np.ascontiguousarray(np.asarray(scales_exp, np.float32)[sl, ::128]),
            "eps": np.ascontiguousarray(np.asarray(eps, np.float32)[:, sl]),
        })
    res = run_bass_kernel_spmd(nc, in_maps, core_ids=list(range(N_CORES)))
    return np.concatenate([res.results[j]["out"] for j in range(N_CORES)], axis=1)


# revision 10
# speedup vs baseline: 116.4219x; 116.4219x over previous
"""LRNetLinear forward on 8 Trainium2 NeuronCores (tensor-parallel over out_features).

Math (per reference):
  3-way softmax over logits [theta_neg, 0, theta_pos] (shift-free: |theta|<=~80 is safe in fp32):
    e_n = exp(theta_neg); e_p = exp(theta_pos); Z = 1 + e_n + e_p; r = 1/Z
    diff = (e_p - e_n) * r
    w_mean = diff * sc                      (sc = scales_exp, constant over 128-wide i-blocks)
    w_var  = (1 - r - diff^2) * sc^2        (since p_pos + p_neg = 1 - r)
  mu = x @ w_mean.T ; s2 = (x*x) @ w_var.T ; out = mu + sqrt(s2 + 1e-8) * eps
  (sqrt via exp(0.5*ln(.)) so every ACT function lives in one table set)

Per-core shard: O_s = 4096/8 = 512 output features. x replicated; out gathered on host.
Matmuls run in float32r (full PE rate, ~1.5e-4 rel err); operands are rounded to
float32r by the PSUM-evacuation copies. x and weights are transposed on-chip with
PE identity-matmul transposes (contraction dim must sit on partitions).
"""
import sys

if "/opt/trn_rl_repo" not in sys.path:
    sys.path.insert(0, "/opt/trn_rl_repo")

import numpy as np

import concourse.bass as bass
import concourse.bacc as bacc
import concourse.mybir as mybir
import concourse.tile as tile
from concourse.bass_utils import run_bass_kernel_spmd
from concourse.masks import make_identity

N_CORES = 8
B = 4096
I = 4096
O = 4096
OS = O // N_CORES          # 512 out features per core
KT = I // 128              # 32 contraction tiles
BT = B // 128              # 32 token tiles
OJ = OS // 128             # 4 o-tiles in weight prep
F32 = mybir.dt.float32
F32R = mybir.dt.float32r
PSUM = bass.MemorySpace.PSUM

_CACHE = {}


def build():
    AF = mybir.ActivationFunctionType
    OP = mybir.AluOpType
    nc = bacc.Bacc("TRN2", target_bir_lowering=False, debug=False, num_devices=N_CORES)
    x_d = nc.dram_tensor("x", [B, I], F32R, kind="ExternalInput").ap()
    tn_d = nc.dram_tensor("tn", [OS, I], F32, kind="ExternalInput").ap()
    tp_d = nc.dram_tensor("tp", [OS, I], F32, kind="ExternalInput").ap()
    scs_d = nc.dram_tensor("scs", [OS, KT], F32, kind="ExternalInput").ap()
    eps_d = nc.dram_tensor("eps", [B, OS], F32, kind="ExternalInput").ap()
    out_d = nc.dram_tensor("out", [B, OS], F32, kind="ExternalOutput").ap()

    with tile.TileContext(nc) as tc:
        with tc.tile_pool(name="const", bufs=1) as cp:
            ident = cp.tile([128, 128], F32)
            make_identity(nc, ident)
            identr = cp.tile([128, 128], F32R)
            nc.vector.tensor_copy(identr, ident)
            # scs[o, k] -> [p, j, k] with o = j*128 + p
            scs_t = cp.tile([128, OJ, KT], F32)
            nc.sync.dma_start(scs_t, scs_d.rearrange("(j p) k -> p j k", p=128))
            sc2 = cp.tile([128, OJ, KT], F32)
            nc.scalar.square(sc2, scs_t)
            nsc2 = cp.tile([128, OJ, KT], F32)
            nc.vector.tensor_scalar_mul(nsc2, sc2, -1.0)
            b1e8 = cp.tile([128, 1], F32)
            nc.vector.memset(b1e8, 1e-8)
            # transposed weights [i_in, k, o], rounded to f32r at evacuation
            wmT = cp.tile([128, KT, OS], F32R)
            wvT = cp.tile([128, KT, OS], F32R)

            # ---------------- weight prep ----------------
            CH = 1024            # i-chunk
            NCH = I // CH        # 4 chunks per o-tile
            SB = CH // 128       # 8 sub-blocks per chunk
            with (
                tc.tile_pool(name="wprep", bufs=2) as wp,
                tc.tile_pool(name="wpsum", bufs=2, space=PSUM) as wps,
            ):
                for j in range(OJ):
                    for c in range(NCH):
                        i0 = c * CH
                        tn_t = wp.tile([128, CH], F32, tag="tn", bufs=2, name="tn_t")
                        nc.sync.dma_start(tn_t, tn_d[128 * j:128 * (j + 1), i0:i0 + CH])
                        tp_t = wp.tile([128, CH], F32, tag="tp", bufs=2, name="tp_t")
                        nc.sync.dma_start(tp_t, tp_d[128 * j:128 * (j + 1), i0:i0 + CH])
                        en = wp.tile([128, CH], F32, tag="en", bufs=1, name="en")
                        nc.scalar.activation(en, tn_t, AF.Exp)
                        ep = wp.tile([128, CH], F32, tag="ep", bufs=1, name="ep")
                        nc.scalar.activation(ep, tp_t, AF.Exp)
                        Z = wp.tile([128, CH], F32, tag="Z", bufs=1, name="Z")
                        nc.vector.scalar_tensor_tensor(Z, ep, 1.0, en, op0=OP.add, op1=OP.add)
                        d = wp.tile([128, CH], F32, tag="d", bufs=1, name="d")
                        nc.vector.tensor_sub(d, ep, en)
                        v = wp.tile([128, CH], F32, tag="v", bufs=1, name="v")
                        nc.scalar.activation(v, Z, AF.Ln)
                        r = wp.tile([128, CH], F32, tag="r", bufs=1, name="r")
                        nc.scalar.activation(r, v, AF.Exp, scale=-1.0)
                        diff = wp.tile([128, CH], F32, tag="diff", bufs=1, name="diff")
                        nc.vector.tensor_mul(diff, d, r)
                        wm = wp.tile([128, CH], F32, tag="wm", bufs=1, name="wm")
                        for s in range(SB):
                            k = (i0 + 128 * s) // 128
                            nc.vector.tensor_scalar_mul(
                                wm[:, 128 * s:128 * (s + 1)],
                                diff[:, 128 * s:128 * (s + 1)],
                                scs_t[:, j, k:k + 1],
                            )
                        d2 = wp.tile([128, CH], F32, tag="d2", bufs=1, name="d2")
                        nc.scalar.square(d2, diff)
                        q = wp.tile([128, CH], F32, tag="q", bufs=1, name="q")
                        nc.vector.tensor_add(q, d2, r)
                        wv = wp.tile([128, CH], F32, tag="wv", bufs=1, name="wv")
                        for s in range(SB):
                            k = (i0 + 128 * s) // 128
                            # (q * -sc^2) + sc^2 = sc^2 * (1 - q)
                            nc.vector.tensor_scalar(
                                wv[:, 128 * s:128 * (s + 1)],
                                q[:, 128 * s:128 * (s + 1)],
                                nsc2[:, j, k:k + 1],
                                sc2[:, j, k:k + 1],
                                op0=OP.mult,
                                op1=OP.add,
                            )
                        for g in range(SB // 4):
                            k0 = i0 // 128 + 4 * g
                            pm = wps.tile([128, 4, 128], F32, tag="pm", name="pm")
                            pv = wps.tile([128, 4, 128], F32, tag="pv", name="pv")
                            for s4 in range(4):
                                s = 4 * g + s4
                                nc.tensor.transpose(pm[:, s4, :], wm[:, 128 * s:128 * (s + 1)], ident)
                                nc.tensor.transpose(pv[:, s4, :], wv[:, 128 * s:128 * (s + 1)], ident)
                            nc.vector.tensor_copy(wmT[:, k0:k0 + 4, 128 * j:128 * (j + 1)], pm)
                            nc.scalar.copy(wvT[:, k0:k0 + 4, 128 * j:128 * (j + 1)], pv)

            # ---------------- main loop over token tiles ----------------
            XCH = 1024
            NXCH = I // XCH
            with (
                tc.tile_pool(name="xnat", bufs=4) as xnp,
                tc.tile_pool(name="xtp", bufs=8) as xtp,
                tc.tile_pool(name="x2p", bufs=8) as x2p,
                tc.tile_pool(name="epsp", bufs=2) as epp,
                tc.tile_pool(name="epip", bufs=4) as eip,
                tc.tile_pool(name="xpsum", bufs=3, space=PSUM) as xps,
                tc.tile_pool(name="opsum", bufs=2, space=PSUM) as ops,
            ):
                def load_x(t):
                    tiles = []
                    for h in range(NXCH):
                        xn = xnp.tile([128, XCH], F32R, tag="xn", name="xn")
                        nc.sync.dma_start(xn, x_d[128 * t:128 * (t + 1), XCH * h:XCH * (h + 1)])
                        tiles.append(xn)
                    return tiles

                def make_pack(xn_tiles, g):
                    # pack g: i in [512g, 512g+512) -> xT/x2T [128, 4, 128] (f32r)
                    h, off = divmod(512 * g, XCH)
                    ps = xps.tile([128, 4, 128], F32R, tag="xp", name="ps")
                    for s4 in range(4):
                        nc.tensor.transpose(
                            ps[:, s4, :], xn_tiles[h][:, off + 128 * s4:off + 128 * (s4 + 1)], identr
                        )
                    xt_g = xtp.tile([128, 4, 128], F32R, tag="xt", name="xt_g")
                    nc.vector.tensor_copy(xt_g, ps)
                    x2_g = x2p.tile([128, 4, 128], F32R, tag="x2", name="x2_g")
                    nc.scalar.square(x2_g, ps)
                    return xt_g, x2_g

                xn = load_x(0)
                packs = [make_pack(xn, g) for g in range(8)]
                for t in range(BT):
                    eps_t = epp.tile([128, OS], F32, tag="ep", name="eps_t")
                    nc.sync.dma_start(eps_t, eps_d[128 * t:128 * (t + 1), :])
                    xn_next = load_x(t + 1) if t + 1 < BT else None
                    pmu = ops.tile([128, OS], F32, tag="pm", name="pmu")
                    pvar = ops.tile([128, OS], F32, tag="pv", name="pvar")
                    next_packs = [None] * 8
                    for k in range(KT):
                        g, s4 = divmod(k, 4)
                        xt_g, x2_g = packs[g]
                        nc.tensor.matmul(pmu, xt_g[:, s4, :], wmT[:, k, :],
                                         start=(k == 0), stop=(k == KT - 1))
                        nc.tensor.matmul(pvar, x2_g[:, s4, :], wvT[:, k, :],
                                         start=(k == 0), stop=(k == KT - 1))
                        if s4 == 3 and xn_next is not None:
                            next_packs[g] = make_pack(xn_next, g)
                    u = eip.tile([128, OS], F32, tag="epi", name="u")
                    nc.scalar.activation(u, pvar, AF.Ln, bias=b1e8)
                    sig = eip.tile([128, OS], F32, tag="epi", name="sig")
                    nc.scalar.activation(sig, u, AF.Exp, scale=0.5)
                    prod = eip.tile([128, OS], F32, tag="epi", name="prod")
                    nc.vector.tensor_mul(prod, sig, eps_t)
                    outt = eip.tile([128, OS], F32, tag="epi", name="outt")
                    nc.vector.tensor_add(outt, pmu, prod)
                    nc.sync.dma_start(out_d[128 * t:128 * (t + 1), :], outt)
                    if xn_next is not None:
                        packs = next_packs

    nc.compile()
    return nc


def _get_nc():
    if "nc" not in _CACHE:
        _CACHE["nc"] = build()
    return _CACHE["nc"]


def kernel(x, theta_neg, theta_pos, scales_exp, eps):
    nc = _get_nc()
    x = np.ascontiguousarray(np.asarray(x, np.float32))
    in_maps = []
    for j in range(N_CORES):
        sl = slice(OS * j, OS * (j + 1))
        in_maps.append({
            "x": x,
            "tn": np.ascontiguousarray(np.asarray(theta_neg, np.float32)[sl]),
            "tp": np.ascontiguousarray(np.asarray(theta_pos, np.float32)[sl]),
            "scs": np.ascontiguousarray(np.asarray(scales_exp, np.float32)[sl, ::128]),
            "eps": np.ascontiguousarray(np.asarray(eps, np.float32)[:, sl]),
        })
    res = run_bass_kernel_spmd(nc, in_maps, core_ids=list(range(N_CORES)))
    return np.concatenate([res.results[j]["out"] for j in range(N_CORES)], axis=1)
